# revision 6
# baseline (speedup 1.0000x reference)
"""Node2Node supervised-contrastive loss on 8 Trainium2 NeuronCores.

Strategy (anchor-sharded, per sharding hint):
  - 1024 anchors split 128 per core (8 cores).
  - Device layout: partition p = local anchor p; free slots c = 0..703:
    cols 0..199 = positives, 200..699 = negatives, 700..703 = padding
    (pad gathers the anchor's own row; masked before exp).
  - Per 8-slot block: indirect-DMA row gather [128, 8, 256] f32 from x,
    fused dot (DVE tensor_tensor_reduce vs. raw anchor rows) and
    sum-of-squares (ACT Square + accum) per 256-elem row.
  - Finisher: rnorm = 1/sqrt(max(ssq, eps^2)); sim/T = dot * rnorm_p *
    (10 * rnorm_a); mask pads; exp; segmented sums (pos / neg cols);
    per-anchor loss = -(1/P) * (ln num - ln den); DMA out [128] f32.
  - Host sums the 1024 per-anchor losses.
"""
from contextlib import ExitStack

import numpy as np

import jax
from jax.sharding import Mesh, PartitionSpec, NamedSharding
from jax.experimental.shard_map import shard_map

import concourse.bass as bass
import concourse.tile as tile
from concourse import bacc, mybir, bass2jax

N_CORES = 8
N_NODES, D = 262144, 256
NUM_ANCHORS = 1024
P_PER_ANCHOR = 200
N_PER_ANCHOR = 500
TEMP = 0.1
EPS = 1e-8

A_LOC = NUM_ANCHORS // N_CORES        # 128 anchors per core
C_SLOTS = 704                          # 200 pos + 500 neg + 4 pad
BLK = 8                                # slots per gather tile
N_BLKS = C_SLOTS // BLK                # 88


class SpmdRunner:
    """jit/shard_map wrapper over a compiled Bass module with cached
    device-resident inputs (mirrors bass2jax.run_bass_via_pjrt).

    replicated: set of input names whose value is identical on all cores;
    they are uploaded once with a replicated sharding instead of an
    8x-concatenated copy.
    """

    def __init__(self, nc, replicated=()):
        bass2jax.install_neuronx_cc_hook()
        self.nc = nc
        self.replicated = set(replicated)
        in_names, out_names, out_avals, zeros = [], [], [], []
        part_name = nc.partition_id_tensor.name if nc.partition_id_tensor else None
        for alloc in nc.m.functions[0].allocations:
            if not isinstance(alloc, mybir.MemoryLocationSet):
                continue
            name = alloc.memorylocations[0].name
            if alloc.kind == "ExternalInput":
                if name != part_name:
                    in_names.append(name)
            elif alloc.kind == "ExternalOutput":
                out_names.append(name)
                shape = tuple(alloc.tensor_shape)
                dtype = mybir.dt.np(alloc.dtype)
                out_avals.append(jax.core.ShapedArray(shape, dtype))
                zeros.append(np.zeros(shape, dtype))
        self.in_names, self.out_names = in_names, out_names
        self.n_params = len(in_names)
        all_in_names = in_names + out_names
        if part_name is not None:
            all_in_names.append(part_name)

        def _body(*args):
            operands = list(args)
            if part_name is not None:
                operands.append(bass2jax.partition_id_tensor())
            return tuple(bass2jax._bass_exec_p.bind(
                *operands,
                out_avals=tuple(out_avals),
                in_names=tuple(all_in_names),
                out_names=tuple(out_names),
                lowering_input_output_aliases=(),
                sim_require_finite=True,
                sim_require_nnan=True,
                nc=nc,
            ))

        devices = jax.devices()[:N_CORES]
        self.mesh = Mesh(np.asarray(devices), ("core",))
        in_specs = tuple(
            PartitionSpec() if n in self.replicated else PartitionSpec("core")
            for n in in_names
        ) + (PartitionSpec("core"),) * len(out_names)
        self.sharded = jax.jit(
            shard_map(_body, mesh=self.mesh,
                      in_specs=in_specs,
                      out_specs=(PartitionSpec("core"),) * len(out_names),
                      check_rep=False),
            keep_unused=True,
        )
        sh = NamedSharding(self.mesh, PartitionSpec("core"))
        self.dev_zeros = [
            jax.device_put(np.zeros((N_CORES * z.shape[0], *z.shape[1:]), z.dtype), sh)
            for z in zeros
        ]
        self.out_avals = out_avals
        self._input_cache = {}

    def put_inputs(self, in_maps, cache_key=None):
        """Concat (or replicate) per-core inputs and device_put."""
        if cache_key is not None and cache_key in self._input_cache:
            return self._input_cache[cache_key]
        sh = NamedSharding(self.mesh, PartitionSpec("core"))
        sh_rep = NamedSharding(self.mesh, PartitionSpec())
        arrs = []
        for name in self.in_names:
            if name in self.replicated:
                arrs.append(jax.device_put(np.asarray(in_maps[0][name]), sh_rep))
            else:
                cat = np.concatenate([np.asarray(m[name]) for m in in_maps], axis=0)
                arrs.append(jax.device_put(cat, sh))
        jax.block_until_ready(arrs)
        if cache_key is not None:
            self._input_cache[cache_key] = arrs
        return arrs

    def run(self, dev_inputs):
        outs = self.sharded(*dev_inputs, *self.dev_zeros)
        jax.block_until_ready(outs)
        return outs

    def fetch(self, outs):
        res = []
        for c in range(N_CORES):
            d = {}
            for i, name in enumerate(self.out_names):
                d[name] = np.asarray(outs[i]).reshape(
                    N_CORES, *self.out_avals[i].shape)[c]
            res.append(d)
        return res


def _build():
    nc = bacc.Bacc("TRN2", target_bir_lowering=False, debug=False, num_devices=N_CORES)
    x_ap = nc.dram_tensor("x", [N_NODES, D], mybir.dt.float32, kind="ExternalInput").ap()
    idx_ap = nc.dram_tensor("idx", [128, C_SLOTS], mybir.dt.int32, kind="ExternalInput").ap()
    loss_ap = nc.dram_tensor("loss", [128, 1], mybir.dt.float32, kind="ExternalOutput").ap()

    f32 = mybir.dt.float32
    AF = mybir.ActivationFunctionType

    with tile.TileContext(nc) as tc, ExitStack() as ctx:
        nc_ = tc.nc
        gpool = ctx.enter_context(tc.tile_pool(name="g", bufs=6))
        state = ctx.enter_context(tc.tile_pool(name="state", bufs=1))
        scratch = ctx.enter_context(tc.tile_pool(name="scr", bufs=2))

        idx_tile = state.tile([128, C_SLOTS], mybir.dt.int32)
        nc_.sync.dma_start(out=idx_tile[:], in_=idx_ap[:])

        # anchor rows (raw): gather via pad column 700
        anchor_tile = state.tile([128, D], f32)
        nc_.gpsimd.indirect_dma_start(
            out=anchor_tile[:], out_offset=None, in_=x_ap[:],
            in_offset=bass.IndirectOffsetOnAxis(ap=idx_tile[:, 700:701], axis=0),
        )

        dots = state.tile([128, C_SLOTS], f32)
        ssq = state.tile([128, C_SLOTS + 1], f32)   # col 704 = anchor ssq

        asq_scr = scratch.tile([128, D], f32, tag="sq_scr")
        nc_.scalar.square(asq_scr[:], anchor_tile[:])
        nc_.vector.tensor_reduce(
            out=ssq[:, C_SLOTS:C_SLOTS + 1], in_=asq_scr[:],
            axis=mybir.AxisListType.X, op=mybir.AluOpType.add,
        )

        for b in range(N_BLKS):
            g = gpool.tile([128, BLK, D], f32)
            for j in range(BLK):
                c = b * BLK + j
                nc_.gpsimd.indirect_dma_start(
                    out=g[:, j, :], out_offset=None, in_=x_ap[:],
                    in_offset=bass.IndirectOffsetOnAxis(ap=idx_tile[:, c:c + 1], axis=0),
                )
            # dots: per-slot multiply into prod tile, one fused reduce per tile
            prod = scratch.tile([128, BLK, D], f32, tag="prod")
            for j in range(BLK):
                nc_.vector.tensor_mul(prod[:, j, :], g[:, j, :], anchor_tile[:])
            nc_.vector.tensor_reduce(
                out=dots[:, b * BLK:(b + 1) * BLK], in_=prod[:],
                axis=mybir.AxisListType.X, op=mybir.AluOpType.add,
            )
            # row sum-of-squares: ACT square (full tile), one reduce per tile
            sq = scratch.tile([128, BLK, D], f32, tag="sq")
            nc_.scalar.square(sq[:], g[:])
            nc_.vector.tensor_reduce(
                out=ssq[:, b * BLK:(b + 1) * BLK], in_=sq[:],
                axis=mybir.AxisListType.X, op=mybir.AluOpType.add,
            )

        # ---- finisher ----
        rn = state.tile([128, C_SLOTS + 1], f32)
        nc_.vector.tensor_scalar_max(rn[:], ssq[:], EPS * EPS)
        nc_.scalar.activation(out=rn[:], in_=rn[:], func=AF.Sqrt)
        nc_.vector.reciprocal(out=rn[:], in_=rn[:])

        scale_a = state.tile([128, 1], f32)
        nc_.vector.tensor_scalar_mul(scale_a[:], rn[:, C_SLOTS:C_SLOTS + 1], 1.0 / TEMP)

        simt = state.tile([128, C_SLOTS], f32)
        nc_.vector.tensor_mul(simt[:], dots[:], rn[:, 0:C_SLOTS])
        nc_.vector.tensor_scalar_mul(simt[:], simt[:], scale_a[:, 0:1])
        nc_.vector.memset(simt[:, 700:704], -1e30)

        ex = state.tile([128, C_SLOTS], f32)
        nc_.scalar.activation(out=ex[:], in_=simt[:], func=AF.Exp)

        num = state.tile([128, 1], f32)
        nc_.vector.tensor_reduce(
            out=num[:], in_=ex[:, 0:P_PER_ANCHOR], axis=mybir.AxisListType.X,
            op=mybir.AluOpType.add,
        )
        negsum = state.tile([128, 1], f32)
        nc_.vector.tensor_reduce(
            out=negsum[:], in_=ex[:, P_PER_ANCHOR:C_SLOTS], axis=mybir.AxisListType.X,
            op=mybir.AluOpType.add,
        )
        nd = state.tile([128, 2], f32)
        nc_.vector.tensor_copy(out=nd[:, 0:1], in_=num[:])
        nc_.vector.tensor_add(nd[:, 1:2], negsum[:], num[:])
        lnd = state.tile([128, 2], f32)
        nc_.scalar.activation(out=lnd[:], in_=nd[:], func=AF.Ln)
        lt = state.tile([128, 1], f32)
        nc_.vector.tensor_sub(lt[:], lnd[:, 0:1], lnd[:, 1:2])
        nc_.vector.tensor_scalar_mul(lt[:], lt[:], -1.0 / P_PER_ANCHOR)
        nc_.sync.dma_start(out=loss_ap[:], in_=lt[:])

    nc.compile()
    return nc


def _build_for_sim():
    """Fresh nc for timeline simulation (kept separate from the runner's)."""
    return _build()


_RUNNER = None


def _get_runner():
    global _RUNNER
    if _RUNNER is None:
        _RUNNER = SpmdRunner(_build(), replicated={"x"})
    return _RUNNER


def _make_in_maps(x, anchor_idx, pos_idx, neg_idx):
    in_maps = []
    for k in range(N_CORES):
        sl = slice(k * A_LOC, (k + 1) * A_LOC)
        idx = np.empty((A_LOC, C_SLOTS), dtype=np.int32)
        idx[:, 0:P_PER_ANCHOR] = pos_idx[sl]
        idx[:, P_PER_ANCHOR:P_PER_ANCHOR + N_PER_ANCHOR] = neg_idx[sl]
        idx[:, 700:704] = np.asarray(anchor_idx[sl], dtype=np.int32)[:, None]
        in_maps.append({"x": x, "idx": idx})
    return in_maps


def kernel(x, anchor_idx, pos_idx, neg_idx):
    x = np.ascontiguousarray(np.asarray(x, dtype=np.float32))
    anchor_idx = np.asarray(anchor_idx).astype(np.int64)
    pos_idx = np.asarray(pos_idx).astype(np.int64)
    neg_idx = np.asarray(neg_idx).astype(np.int64)

    runner = _get_runner()
    in_maps = _make_in_maps(x, anchor_idx, pos_idx, neg_idx)
    dev = runner.put_inputs(in_maps, cache_key=(id(x), id(pos_idx)))
    outs = runner.run(dev)
    res = runner.fetch(outs)
    total = np.float32(0.0)
    for k in range(N_CORES):
        total += np.sum(res[k]["loss"].astype(np.float32))
    return np.float32(total)


# revision 7
# speedup vs baseline: 1.3678x; 1.3678x over previous
"""Node2Node supervised-contrastive loss on 8 Trainium2 NeuronCores.

Strategy (anchor-sharded per the sharding hint, hybrid gather):
  - 1024 anchors split 128 per core. Device layout: partition p = local
    anchor p, 704 free-dim slots per anchor (200 pos + 500 neg + 4 pad).
  - Each anchor's slots are permuted host-side (sums are order-invariant)
    so a maximal prefix of columns is "window-pure": all 128 anchors'
    indices in that column fall in the same 32768-row window of x. Pure
    columns are gathered with the TIE-accelerated int16 dma_gather
    (<=1024 rows per instruction - larger wedges the SWDGE ring);
    leftover mixed columns use one indirect_dma_start per column
    ([P,1] int32 offsets, the only HW-supported indirect form).
  - Per gathered row (raw f32): dot vs raw anchor row (DVE mul + reduce)
    and sum-of-squares (ACT Square + accum_out). Then
    sim/T = dot * rsqrt(ssq_p) * (rsqrt(ssq_a)/T), exp on ACT, and
    pos/neg membership masks (host-built, follow the permutation) give
    numerator/denominator via two masked reduces. Per-anchor loss
    -(1/200)*(ln num - ln den) is DMA'd out; host sums 1024 values.
  - x is uploaded once, replicated to all 8 cores; the program is
    specialized at call time to the actual index distribution (the
    window-pure column budget), then compiled and cached.
"""
from contextlib import ExitStack

import numpy as np

import jax
from jax.sharding import Mesh, PartitionSpec, NamedSharding
from jax.experimental.shard_map import shard_map

import concourse.bass as bass
import concourse.tile as tile
from concourse import bacc, mybir, bass2jax

N_CORES = 8
N_NODES, D = 262144, 256
NUM_ANCHORS = 1024
P_PER = 200
N_PER = 500
TEMP = 0.1
EPS = 1e-8

A_LOC = NUM_ANCHORS // N_CORES
C_SLOTS = 704           # 200 pos + 500 neg + 4 pad
WIN = 32768             # int16-addressable row window for dma_gather
N_WIN = N_NODES // WIN
GMAX = 8                # dma_gather columns per instruction (1024 rows)


class SpmdRunner:
    """jit/shard_map wrapper over a compiled Bass module with cached
    device-resident inputs (mirrors bass2jax.run_bass_via_pjrt)."""

    def __init__(self, nc, replicated=()):
        bass2jax.install_neuronx_cc_hook()
        self.nc = nc
        self.replicated = set(replicated)
        in_names, out_names, out_avals, zeros = [], [], [], []
        part_name = nc.partition_id_tensor.name if nc.partition_id_tensor else None
        for alloc in nc.m.functions[0].allocations:
            if not isinstance(alloc, mybir.MemoryLocationSet):
                continue
            name = alloc.memorylocations[0].name
            if alloc.kind == "ExternalInput":
                if name != part_name:
                    in_names.append(name)
            elif alloc.kind == "ExternalOutput":
                out_names.append(name)
                shape = tuple(alloc.tensor_shape)
                dtype = mybir.dt.np(alloc.dtype)
                out_avals.append(jax.core.ShapedArray(shape, dtype))
                zeros.append(np.zeros(shape, dtype))
        self.in_names, self.out_names = in_names, out_names
        self.n_params = len(in_names)
        all_in_names = in_names + out_names
        if part_name is not None:
            all_in_names.append(part_name)

        def _body(*args):
            operands = list(args)
            if part_name is not None:
                operands.append(bass2jax.partition_id_tensor())
            return tuple(bass2jax._bass_exec_p.bind(
                *operands,
                out_avals=tuple(out_avals),
                in_names=tuple(all_in_names),
                out_names=tuple(out_names),
                lowering_input_output_aliases=(),
                sim_require_finite=True,
                sim_require_nnan=True,
                nc=nc,
            ))

        devices = jax.devices()[:N_CORES]
        self.mesh = Mesh(np.asarray(devices), ("core",))
        in_specs = tuple(
            PartitionSpec() if n in self.replicated else PartitionSpec("core")
            for n in in_names
        ) + (PartitionSpec("core"),) * len(out_names)
        self.sharded = jax.jit(
            shard_map(_body, mesh=self.mesh,
                      in_specs=in_specs,
                      out_specs=(PartitionSpec("core"),) * len(out_names),
                      check_rep=False),
            keep_unused=True,
        )
        sh = NamedSharding(self.mesh, PartitionSpec("core"))
        self.dev_zeros = [
            jax.device_put(np.zeros((N_CORES * z.shape[0], *z.shape[1:]), z.dtype), sh)
            for z in zeros
        ]
        self.out_avals = out_avals
        self._input_cache = {}

    def put_inputs(self, in_maps, cache_key=None):
        if cache_key is not None and cache_key in self._input_cache:
            return self._input_cache[cache_key]
        sh = NamedSharding(self.mesh, PartitionSpec("core"))
        sh_rep = NamedSharding(self.mesh, PartitionSpec())
        arrs = []
        for name in self.in_names:
            if name in self.replicated:
                arrs.append(jax.device_put(np.asarray(in_maps[0][name]), sh_rep))
            else:
                cat = np.concatenate([np.asarray(m[name]) for m in in_maps], axis=0)
                arrs.append(jax.device_put(cat, sh))
        jax.block_until_ready(arrs)
        if cache_key is not None:
            self._input_cache[cache_key] = arrs
        return arrs

    def run(self, dev_inputs):
        outs = self.sharded(*dev_inputs, *self.dev_zeros)
        jax.block_until_ready(outs)
        return outs

    def fetch(self, outs):
        res = []
        for c in range(N_CORES):
            d = {}
            for i, name in enumerate(self.out_names):
                d[name] = np.asarray(outs[i]).reshape(
                    N_CORES, *self.out_avals[i].shape)[c]
            res.append(d)
        return res


def plan_layout(anchor_idx, pos_idx, neg_idx):
    """Permute each anchor's 704 slots so the first sum(pure) columns are
    window-blocked uniformly across all 1024 anchors. Returns the pure
    per-window column counts, permuted indices, and pos/valid masks."""
    idx_all = np.concatenate(
        [pos_idx, neg_idx, np.repeat(anchor_idx[:, None], 4, axis=1)], axis=1
    ).astype(np.int64)
    is_pos = np.zeros((NUM_ANCHORS, C_SLOTS), dtype=bool)
    is_pos[:, :P_PER] = True
    is_valid = np.zeros((NUM_ANCHORS, C_SLOTS), dtype=bool)
    is_valid[:, :P_PER + N_PER] = True

    win = (idx_all >> 15).astype(np.int64)
    counts = np.zeros((NUM_ANCHORS, N_WIN), dtype=np.int64)
    for w in range(N_WIN):
        counts[:, w] = (win == w).sum(axis=1)
    pure = counts.min(axis=0)

    perm = np.empty((NUM_ANCHORS, C_SLOTS), dtype=np.int64)
    for a in range(NUM_ANCHORS):
        order, leftovers = [], []
        wslots = [np.nonzero(win[a] == w)[0] for w in range(N_WIN)]
        for w in range(N_WIN):
            take = int(pure[w])
            order.append(wslots[w][:take])
            leftovers.append(wslots[w][take:])
        order.append(np.concatenate(leftovers))
        perm[a] = np.concatenate(order)

    idx_p = np.take_along_axis(idx_all, perm, axis=1)
    posm = np.take_along_axis(is_pos, perm, axis=1).astype(np.float32)
    valm = np.take_along_axis(is_valid, perm, axis=1).astype(np.float32)
    return pure, idx_p, posm, valm


def build_nc(pure):
    n_pure = int(pure.sum())
    n_mixed = C_SLOTS - n_pure
    idx16_cols = 8 * n_pure

    nc = bacc.Bacc("TRN2", target_bir_lowering=False, debug=False,
                   num_devices=N_CORES, dynamic_dma_scratch_size=65536)
    x_ap = nc.dram_tensor("x", [N_NODES, D], mybir.dt.float32, kind="ExternalInput").ap()
    idx16_ap = nc.dram_tensor("idx16", [128, idx16_cols], mybir.dt.int16, kind="ExternalInput").ap()
    idxm_ap = nc.dram_tensor("idxm", [128, n_mixed], mybir.dt.int32, kind="ExternalInput").ap()
    aidx_ap = nc.dram_tensor("aidx", [128, 1], mybir.dt.int32, kind="ExternalInput").ap()
    posm_ap = nc.dram_tensor("posm", [128, C_SLOTS], mybir.dt.float32, kind="ExternalInput").ap()
    valm_ap = nc.dram_tensor("valm", [128, C_SLOTS], mybir.dt.float32, kind="ExternalInput").ap()
    loss_ap = nc.dram_tensor("loss", [128, 1], mybir.dt.float32, kind="ExternalOutput").ap()

    f32 = mybir.dt.float32
    AF = mybir.ActivationFunctionType

    with tile.TileContext(nc) as tc, ExitStack() as ctx:
        nc_ = tc.nc
        gpool = ctx.enter_context(tc.tile_pool(name="g", bufs=3))
        state = ctx.enter_context(tc.tile_pool(name="state", bufs=1))
        scratch = ctx.enter_context(tc.tile_pool(name="scr", bufs=3))

        idx16_tile = state.tile([128, idx16_cols], mybir.dt.int16)
        nc_.sync.dma_start(out=idx16_tile[:], in_=idx16_ap[:])
        idxm_tile = state.tile([128, n_mixed], mybir.dt.int32)
        nc_.sync.dma_start(out=idxm_tile[:], in_=idxm_ap[:])
        aidx_tile = state.tile([128, 1], mybir.dt.int32)
        nc_.sync.dma_start(out=aidx_tile[:], in_=aidx_ap[:])
        posm_tile = state.tile([128, C_SLOTS], f32)
        nc_.sync.dma_start(out=posm_tile[:], in_=posm_ap[:])
        valm_tile = state.tile([128, C_SLOTS], f32)
        nc_.sync.dma_start(out=valm_tile[:], in_=valm_ap[:])

        anchor_tile = state.tile([128, D], f32)
        nc_.gpsimd.indirect_dma_start(
            out=anchor_tile[:], out_offset=None, in_=x_ap[:],
            in_offset=bass.IndirectOffsetOnAxis(ap=aidx_tile[:, 0:1], axis=0),
        )

        tc.strict_bb_all_engine_barrier()

        dots = state.tile([128, C_SLOTS], f32)
        ssq = state.tile([128, C_SLOTS + 1], f32)   # col 704 = anchor ssq

        asq_scr = scratch.tile([128, D], f32, tag="sq")
        nc_.scalar.activation(out=asq_scr[:], in_=anchor_tile[:], func=AF.Square,
                              accum_out=ssq[:, C_SLOTS:C_SLOTS + 1])

        def compute_tile(g, col0, ncols):
            prod = scratch.tile([128, ncols, D], f32, tag="prod")
            for j in range(ncols):
                nc_.vector.tensor_mul(prod[:, j, :], g[:, j, :], anchor_tile[:])
            nc_.vector.tensor_reduce(
                out=dots[:, col0:col0 + ncols], in_=prod[:],
                axis=mybir.AxisListType.X, op=mybir.AluOpType.add)
            for j in range(ncols):
                sq_scr = scratch.tile([128, D], f32, tag="sq")
                nc_.scalar.activation(out=sq_scr[:], in_=g[:, j, :], func=AF.Square,
                                      accum_out=ssq[:, col0 + j:col0 + j + 1])

        # pure columns: dma_gather per <=GMAX-col chunk, per window
        col = 0
        i16 = 0
        for w in range(N_WIN):
            nw = int(pure[w])
            x_win = x_ap[w * WIN:(w + 1) * WIN, :]
            off = 0
            while off < nw:
                ncols = min(GMAX, nw - off)
                g = gpool.tile([128, GMAX, D], f32, tag="g")
                nc_.gpsimd.dma_gather(
                    out_ap=g[:, 0:ncols, :], in_ap=x_win,
                    idxs_ap=idx16_tile[:, i16:i16 + 8 * ncols],
                    num_idxs=128 * ncols, num_idxs_reg=128 * ncols,
                    elem_size=256,
                )
                compute_tile(g, col, ncols)
                col += ncols
                i16 += 8 * ncols
                off += ncols

        tc.strict_bb_all_engine_barrier()

        # mixed columns: one indirect [P,1] gather per column, blocks of 8
        off = 0
        while off < n_mixed:
            ncols = min(8, n_mixed - off)
            g = gpool.tile([128, GMAX, D], f32, tag="g")
            for j in range(ncols):
                nc_.gpsimd.indirect_dma_start(
                    out=g[:, j, :], out_offset=None, in_=x_ap[:],
                    in_offset=bass.IndirectOffsetOnAxis(
                        ap=idxm_tile[:, off + j:off + j + 1], axis=0),
                )
            compute_tile(g, col, ncols)
            col += ncols
            off += ncols
        assert col == C_SLOTS

        # finisher
        rn = state.tile([128, C_SLOTS + 1], f32)
        nc_.vector.tensor_scalar_max(rn[:], ssq[:], EPS * EPS)
        nc_.scalar.activation(out=rn[:], in_=rn[:], func=AF.Sqrt)
        nc_.vector.reciprocal(out=rn[:], in_=rn[:])

        scale_a = state.tile([128, 1], f32)
        nc_.vector.tensor_scalar_mul(scale_a[:], rn[:, C_SLOTS:C_SLOTS + 1], 1.0 / TEMP)

        simt = state.tile([128, C_SLOTS], f32)
        nc_.vector.tensor_mul(simt[:], dots[:], rn[:, 0:C_SLOTS])
        nc_.vector.tensor_scalar_mul(simt[:], simt[:], scale_a[:, 0:1])

        ex = state.tile([128, C_SLOTS], f32)
        nc_.scalar.activation(out=ex[:], in_=simt[:], func=AF.Exp)

        exp_pos = state.tile([128, C_SLOTS], f32)
        nc_.vector.tensor_mul(exp_pos[:], ex[:], posm_tile[:])
        exp_val = state.tile([128, C_SLOTS], f32)
        nc_.vector.tensor_mul(exp_val[:], ex[:], valm_tile[:])

        nd = state.tile([128, 2], f32)
        nc_.vector.tensor_reduce(out=nd[:, 0:1], in_=exp_pos[:],
                                 axis=mybir.AxisListType.X, op=mybir.AluOpType.add)
        nc_.vector.tensor_reduce(out=nd[:, 1:2], in_=exp_val[:],
                                 axis=mybir.AxisListType.X, op=mybir.AluOpType.add)
        lnd = state.tile([128, 2], f32)
        nc_.scalar.activation(out=lnd[:], in_=nd[:], func=AF.Ln)
        lt = state.tile([128, 1], f32)
        nc_.vector.tensor_sub(lt[:], lnd[:, 0:1], lnd[:, 1:2])
        nc_.vector.tensor_scalar_mul(lt[:], lt[:], -1.0 / P_PER)
        nc_.sync.dma_start(out=loss_ap[:], in_=lt[:])

    nc.compile()
    return nc


def make_in_maps(x, pure, idx_p, posm, valm, anchor_idx):
    n_pure = int(pure.sum())
    in_maps = []
    for k in range(N_CORES):
        sl = slice(k * A_LOC, (k + 1) * A_LOC)
        ip = idx_p[sl]
        cols16 = []
        col = 0
        for w in range(N_WIN):
            nw = int(pure[w])
            off = 0
            while off < nw:
                ncols = min(GMAX, nw - off)
                n_idx = 128 * ncols
                logical = (ip[:, col:col + ncols] - (np.int64(w) << 15)).T.reshape(-1)
                wrapped = np.zeros((16, n_idx // 16), dtype=np.int16)
                ar = np.arange(n_idx)
                wrapped[ar % 16, ar // 16] = logical.astype(np.int16)
                cols16.append(np.tile(wrapped, (8, 1)))
                col += ncols
                off += ncols
        idx16 = (np.concatenate(cols16, axis=1) if cols16
                 else np.zeros((128, 0), np.int16))
        in_maps.append({
            "x": x,
            "idx16": idx16,
            "idxm": np.ascontiguousarray(ip[:, n_pure:].astype(np.int32)),
            "aidx": np.ascontiguousarray(anchor_idx[sl].astype(np.int32)[:, None]),
            "posm": np.ascontiguousarray(posm[sl]),
            "valm": np.ascontiguousarray(valm[sl]),
        })
    return in_maps


_RUNNERS = {}   # keyed by tuple(pure): program is layout-specialized
_LAST_NC = None


def _get_runner(pure):
    global _LAST_NC
    key = tuple(int(p) for p in pure)
    if key not in _RUNNERS:
        nc = build_nc(pure)
        _LAST_NC = nc
        _RUNNERS[key] = SpmdRunner(nc, replicated={"x"})
    return _RUNNERS[key]


def kernel(x, anchor_idx, pos_idx, neg_idx):
    x = np.ascontiguousarray(np.asarray(x, dtype=np.float32))
    anchor_idx = np.asarray(anchor_idx).astype(np.int64)
    pos_idx = np.asarray(pos_idx).astype(np.int64)
    neg_idx = np.asarray(neg_idx).astype(np.int64)

    pure, idx_p, posm, valm = plan_layout(anchor_idx, pos_idx, neg_idx)
    runner = _get_runner(pure)
    in_maps = make_in_maps(x, pure, idx_p, posm, valm, anchor_idx)
    dev = runner.put_inputs(in_maps, cache_key=(id(x), id(pos_idx)))
    outs = runner.run(dev)
    res = runner.fetch(outs)
    total = np.float32(0.0)
    for k in range(N_CORES):
        total += np.sum(res[k]["loss"].astype(np.float32))
    return np.float32(total)
